# revision 2
# baseline (speedup 1.0000x reference)
"""MoE block (B=2,S=2048,D=2048,FF=8192,E=16,K=2,C=640) on 8 trn2 cores.

Expert parallelism: 2 experts per core. Each core redundantly computes the
top-2 gate + capacity positions (exact fp32), dispatches its local experts'
tokens via indirect DMA gather, runs the expert FFN in bf16 (fp32 accum),
and scatter-adds gate-weighted rows into a per-core partial output.
Host sums the 8 partials (expert-parallel combine/unshard).
"""
import sys
sys.path.insert(0, "/opt/trn_rl_repo")
import numpy as np
import ml_dtypes

import concourse.bass as bass
import concourse.mybir as mybir
import concourse.tile as tile
from concourse import bacc
from concourse.bass_utils import run_bass_kernel_spmd

F32 = mybir.dt.float32
BF16 = mybir.dt.bfloat16
I32 = mybir.dt.int32
U32 = mybir.dt.uint32
AL = mybir.AluOpType
ACTF = mybir.ActivationFunctionType

B, S, D, FF, E, K = 2, 2048, 2048, 8192, 16, 2
T = B * S                 # 4096 tokens
C = 640                   # per-expert capacity
NB = T // 128             # 32 token blocks
EL = 2                    # local experts per core
NF = FF // 128            # 64 f-tiles
ND = D // 512             # 4 dd chunks
NCT = C // 128            # 5 capacity tiles per expert
NK = D // 128             # 16 contraction tiles of D

_CACHE = {}


def _build_nc():
    nc = bacc.Bacc(None, target_bir_lowering=False, debug=True)

    xt = nc.dram_tensor("xt", [D, T], F32, kind="ExternalInput")
    xb = nc.dram_tensor("xb", [T + 1, D], BF16, kind="ExternalInput")
    wg = nc.dram_tensor("wg", [D, E], F32, kind="ExternalInput")
    w1h = nc.dram_tensor("w1h", [EL, NF, 128, NK, 128], BF16, kind="ExternalInput")
    w2h = nc.dram_tensor("w2h", [EL, ND, NF, 128, 512], BF16, kind="ExternalInput")
    b1h = nc.dram_tensor("b1h", [EL, 128, NF], F32, kind="ExternalInput")
    b2d = nc.dram_tensor("b2d", [EL, D], F32, kind="ExternalInput")
    tokids = nc.dram_tensor("tokids", [128, NB], F32, kind="ExternalInput")
    iota16 = nc.dram_tensor("iota16", [128, E], F32, kind="ExternalInput")
    onescol = nc.dram_tensor("onescol", [128, 1], F32, kind="ExternalInput")
    tri128 = nc.dram_tensor("tri128", [128, 128], F32, kind="ExternalInput")
    identb = nc.dram_tensor("identb", [128, 128], BF16, kind="ExternalInput")
    basev = nc.dram_tensor("basev", [128, 1], F32, kind="ExternalInput")

    slots = nc.dram_tensor("slots", [EL * C, 2], F32)
    exclb = nc.dram_tensor("exclb", [2, NB * E], F32)

    y = nc.dram_tensor("y", [T, D], F32, kind="ExternalOutput")

    with tile.TileContext(nc) as tc:
        with tc.tile_pool(name="consts", bufs=1) as cp:
            wg_sb = cp.tile([128, NK * E], F32)
            for k in range(NK):
                nc.sync.dma_start(wg_sb[:, k * E:(k + 1) * E], wg[k * 128:(k + 1) * 128, :])
            iota_sb = cp.tile([128, E], F32)
            nc.sync.dma_start(iota_sb[:], iota16[:])
            ones_sb = cp.tile([128, 1], F32)
            nc.sync.dma_start(ones_sb[:], onescol[:])
            tri_sb = cp.tile([128, 128], F32)
            nc.sync.dma_start(tri_sb[:], tri128[:])
            idb_sb = cp.tile([128, 128], BF16)
            nc.sync.dma_start(idb_sb[:], identb[:])
            base_sb = cp.tile([128, 1], F32)
            nc.sync.dma_start(base_sb[:], basev[:])
            tok_sb = cp.tile([128, NB], F32)
            nc.sync.dma_start(tok_sb[:], tokids[:])
            b1_sb = [cp.tile([128, NF], F32, tag=f"b1_{e}", name=f"b1_{e}") for e in range(EL)]
            b2_sb = [cp.tile([128, D], F32, tag=f"b2_{e}", name=f"b2_{e}") for e in range(EL)]
            for e in range(EL):
                nc.gpsimd.dma_start(b1_sb[e][:], b1h[e])
                nc.gpsimd.dma_start(b2_sb[e][:], b2d[e:e + 1, :].to_broadcast([128, D]))

            # sentinel-init the slot table: tok=T (zero row of xb), gate=0
            sent = cp.tile([128, 2], F32)
            nc.vector.memset(sent[:, 0:1], float(T))
            nc.vector.memset(sent[:, 1:2], 0.0)
            for n in range(EL * C // 128):
                nc.gpsimd.dma_start(slots[n * 128:(n + 1) * 128, :], sent[:])

            # ---------------- routing (replicated on every core) ----------------
            with tc.tile_pool(name="rout", bufs=1) as rp, \
                 tc.tile_pool(name="psr", bufs=1, space="PSUM") as pr:
                logits = rp.tile([128, NB * E], F32)
                mx_all = rp.tile([128, NB, 8], F32)
                mi_all = rp.tile([128, NB, 8], U32)
                oh0 = rp.tile([128, NB * E], F32)
                oh1 = rp.tile([128, NB * E], F32)

                for b in range(NB):
                    lg = pr.tile([128, E], F32, tag="lg", bufs=2)
                    for k in range(NK):
                        xtt = rp.tile([128, 128], F32, tag="xtg", bufs=4)
                        nc.sync.dma_start(
                            xtt[:], xt[k * 128:(k + 1) * 128, b * 128:(b + 1) * 128])
                        nc.tensor.matmul(lg[:], lhsT=xtt[:], rhs=wg_sb[:, k * E:(k + 1) * E],
                                         start=(k == 0), stop=(k == NK - 1))
                    sl = slice(b * E, (b + 1) * E)
                    nc.vector.tensor_copy(logits[:, sl], lg[:])
                    nc.vector.max(out=mx_all[:, b, :], in_=logits[:, sl])
                    nc.vector.max_index(out=mi_all[:, b, :], in_max=mx_all[:, b, :],
                                        in_values=logits[:, sl])

                i0f = rp.tile([128, NB], F32)
                i1f = rp.tile([128, NB], F32)
                nc.vector.tensor_copy(i0f[:], mi_all[:, :, 0])
                nc.vector.tensor_copy(i1f[:], mi_all[:, :, 1])
                for b in range(NB):
                    sl = slice(b * E, (b + 1) * E)
                    nc.vector.tensor_tensor(out=oh0[:, sl], in0=iota_sb[:],
                                            in1=i0f[:, b:b + 1].to_broadcast([128, E]),
                                            op=AL.is_equal)
                    nc.vector.tensor_tensor(out=oh1[:, sl], in0=iota_sb[:],
                                            in1=i1f[:, b:b + 1].to_broadcast([128, E]),
                                            op=AL.is_equal)

                # gates from top-2 logits: g0 = 1/(1+e^(l1-l0)), g1 = 1-g0
                l0 = mx_all[:, :, 0]
                l1 = mx_all[:, :, 1]
                dte = rp.tile([128, NB], F32)
                nc.vector.tensor_tensor(out=dte[:], in0=l1, in1=l0, op=AL.subtract)
                exd = rp.tile([128, NB], F32)
                nc.scalar.activation(exd[:], dte[:], ACTF.Exp)
                den = rp.tile([128, NB], F32)
                nc.vector.tensor_scalar_add(den[:], exd[:], 1.0)
                g0 = rp.tile([128, NB], F32)
                nc.vector.reciprocal(g0[:], den[:])
                g1 = rp.tile([128, NB], F32)
                nc.vector.tensor_tensor(out=g1[:], in0=exd[:], in1=g0[:], op=AL.mult)

                # per-expert counts per block: countsT[e, b] via ones-matmuls
                cnt_ps = [pr.tile([E, NB], F32, tag=f"cnt{s}", name=f"cnt{s}", bufs=1) for s in range(2)]
                for b in range(NB):
                    sl = slice(b * E, (b + 1) * E)
                    nc.tensor.matmul(cnt_ps[0][:, b:b + 1], lhsT=oh0[:, sl], rhs=ones_sb[:])
                    nc.tensor.matmul(cnt_ps[1][:, b:b + 1], lhsT=oh1[:, sl], rhs=ones_sb[:])
                zer = rp.tile([E, NB], F32)
                nc.vector.memset(zer[:], 0.0)
                incl = [rp.tile([E, NB], F32, tag=f"incl{s}", name=f"incl{s}") for s in range(2)]
                excl = [rp.tile([E, NB], F32, tag=f"excl{s}", name=f"excl{s}") for s in range(2)]
                for s in range(2):
                    nc.vector.tensor_tensor_scan(out=incl[s][:], data0=cnt_ps[s][:],
                                                 data1=zer[:], initial=0.0,
                                                 op0=AL.add, op1=AL.add)
                    nc.vector.tensor_tensor(out=excl[s][:], in0=incl[s][:],
                                            in1=cnt_ps[s][:], op=AL.subtract)
                # slot-1 block offsets additionally shifted by total slot-0 counts
                nc.vector.tensor_tensor(out=excl[1][:], in0=excl[1][:],
                                        in1=incl[0][:, NB - 1:NB].to_broadcast([E, NB]),
                                        op=AL.add)
                for s in range(2):
                    nc.gpsimd.dma_start(
                        exclb[s:s + 1, :].rearrange("o (b e) -> (o e) b", e=E),
                        excl[s][:])
                bc = [rp.tile([128, NB * E], F32, tag=f"bc{s}", name=f"bc{s}") for s in range(2)]
                for s in range(2):
                    nc.gpsimd.dma_start(bc[s][:], exclb[s:s + 1, :].to_broadcast([128, NB * E]))

                # positions: in-block inclusive cumsum (triangular matmul) + block offset
                pos = [rp.tile([128, NB], F32, tag=f"pos{s}", name=f"pos{s}") for s in range(2)]
                for s, oh in enumerate((oh0, oh1)):
                    for b in range(NB):
                        sl = slice(b * E, (b + 1) * E)
                        cu = pr.tile([128, E], F32, tag="cum", bufs=2)
                        nc.tensor.matmul(cu[:], lhsT=tri_sb[:], rhs=oh[:, sl])
                        t1 = rp.tile([128, E], F32, tag="pt", bufs=3)
                        nc.vector.tensor_tensor(out=t1[:], in0=cu[:], in1=bc[s][:, sl], op=AL.add)
                        nc.vector.tensor_tensor(out=t1[:], in0=t1[:], in1=oh[:, sl], op=AL.mult)
                        nc.vector.tensor_reduce(out=pos[s][:, b:b + 1], in_=t1[:],
                                                axis=mybir.AxisListType.X, op=AL.add)
                    nc.vector.tensor_scalar_add(pos[s][:], pos[s][:], -1.0)

                # scatter (token_id, gate) into local slot table
                for s, (idxf, gv) in enumerate(((i0f, g0), (i1f, g1))):
                    offc = rp.tile([128, NB], F32, tag=f"off{s}")
                    nc.vector.tensor_scalar_mul(offc[:], idxf[:], float(C))
                    nc.vector.tensor_tensor(out=offc[:], in0=offc[:], in1=pos[s][:], op=AL.add)
                    nc.vector.tensor_scalar_sub(offc[:], offc[:], base_sb[:, 0:1])
                    m1 = rp.tile([128, NB], F32, tag="m1s")
                    m2 = rp.tile([128, NB], F32, tag="m2s")
                    nc.vector.tensor_scalar(m1[:], offc[:], 0.0, None, op0=AL.is_ge)
                    nc.vector.tensor_scalar(m2[:], offc[:], float(EL * C), None, op0=AL.is_lt)
                    nc.vector.tensor_tensor(out=m1[:], in0=m1[:], in1=m2[:], op=AL.mult)
                    nc.vector.tensor_scalar(m2[:], pos[s][:], float(C), None, op0=AL.is_lt)
                    nc.vector.tensor_tensor(out=m1[:], in0=m1[:], in1=m2[:], op=AL.mult)
                    nc.vector.tensor_tensor(out=offc[:], in0=offc[:], in1=m1[:], op=AL.mult)
                    nc.vector.tensor_scalar(m2[:], m1[:], -2.0e9, 2.0e9, op0=AL.mult, op1=AL.add)
                    nc.vector.tensor_tensor(out=offc[:], in0=offc[:], in1=m2[:], op=AL.add)
                    offi = rp.tile([128, NB], I32, tag=f"offi{s}")
                    nc.vector.tensor_copy(offi[:], offc[:])
                    pay = rp.tile([128, NB, 2], F32, tag=f"pay{s}")
                    nc.vector.tensor_copy(pay[:, :, 0], tok_sb[:])
                    nc.vector.tensor_copy(pay[:, :, 1], gv[:])
                    for b in range(NB):
                        nc.gpsimd.indirect_dma_start(
                            out=slots[:, :],
                            out_offset=bass.IndirectOffsetOnAxis(ap=offi[:, b:b + 1], axis=0),
                            in_=pay[:, b, :], in_offset=None,
                            bounds_check=EL * C - 1, oob_is_err=False)

            # ---------------- expert FFN + combine ----------------
            with tc.tile_pool(name="ffn", bufs=1) as fp, \
                 tc.tile_pool(name="psf", bufs=1, space="PSUM") as pf:
                for e in range(EL):
                    slot_sb = [fp.tile([128, 2], F32, tag=f"slot{ct}", name=f"slot{ct}", bufs=2)
                               for ct in range(NCT)]
                    tok_i = [fp.tile([128, 1], I32, tag=f"toki{ct}", name=f"toki{ct}", bufs=2)
                             for ct in range(NCT)]
                    xte = [fp.tile([128, C], BF16, tag=f"xte{k}", name=f"xte{k}", bufs=1)
                           for k in range(NK)]
                    for ct in range(NCT):
                        r0 = e * C + ct * 128
                        nc.gpsimd.dma_start(slot_sb[ct][:], slots[r0:r0 + 128, :])
                        nc.vector.tensor_copy(tok_i[ct][:], slot_sb[ct][:, 0:1])
                        xg = fp.tile([128, D], BF16, tag="xg", bufs=3)
                        nc.gpsimd.indirect_dma_start(
                            out=xg[:], out_offset=None, in_=xb[:, :],
                            in_offset=bass.IndirectOffsetOnAxis(ap=tok_i[ct][:, :1], axis=0))
                        for k in range(NK):
                            tp = pf.tile([128, 128], BF16, tag="ptr", bufs=1)
                            nc.tensor.transpose(out=tp[:], in_=xg[:, k * 128:(k + 1) * 128],
                                                identity=idb_sb[:])
                            nc.vector.tensor_copy(xte[k][:, ct * 128:(ct + 1) * 128], tp[:])

                    # mm1 + GELU: hT[f] = gelu(W1[:,f].T @ X.T + b1[f])
                    ht = [fp.tile([128, C], BF16, tag=f"ht{f}", name=f"ht{f}", bufs=1) for f in range(NF)]
                    for f in range(NF):
                        w1c = fp.tile([128, NK * 128], BF16, tag="w1c", bufs=3)
                        nc.sync.dma_start(w1c[:], w1h[e, f])
                        psA = pf.tile([128, 320], F32, tag="m1", bufs=2)
                        psB = pf.tile([128, 320], F32, tag="m1", bufs=2)
                        for k in range(NK):
                            lw = w1c[:, k * 128:(k + 1) * 128]
                            nc.tensor.matmul(psA[:], lhsT=lw, rhs=xte[k][:, 0:320],
                                             start=(k == 0), stop=(k == NK - 1))
                            nc.tensor.matmul(psB[:], lhsT=lw, rhs=xte[k][:, 320:640],
                                             start=(k == 0), stop=(k == NK - 1))
                        nc.scalar.activation(ht[f][:, 0:320], psA[:], ACTF.Gelu,
                                             bias=b1_sb[e][:, f:f + 1])
                        nc.scalar.activation(ht[f][:, 320:640], psB[:], ACTF.Gelu,
                                             bias=b1_sb[e][:, f:f + 1])

                    # mm2 + bias + gate-scale + scatter-add into y
                    for dd in range(ND):
                        psY = [pf.tile([128, 512], F32, tag=f"m2_{ct}", name=f"m2_{ct}", bufs=1)
                               for ct in range(NCT)]
                        for f in range(NF):
                            w2c = fp.tile([128, 512], BF16, tag="w2c", bufs=3)
                            nc.sync.dma_start(w2c[:], w2h[e, dd, f])
                            for ct in range(NCT):
                                nc.tensor.matmul(psY[ct][:],
                                                 lhsT=ht[f][:, ct * 128:(ct + 1) * 128],
                                                 rhs=w2c[:],
                                                 start=(f == 0), stop=(f == NF - 1))
                        for ct in range(NCT):
                            ye = fp.tile([128, 512], F32, tag="ye", bufs=3)
                            nc.vector.tensor_tensor(
                                out=ye[:], in0=psY[ct][:],
                                in1=b2_sb[e][:, dd * 512:(dd + 1) * 512], op=AL.add)
                            nc.vector.tensor_scalar_mul(ye[:], ye[:], slot_sb[ct][:, 1:2])
                            nc.gpsimd.indirect_dma_start(
                                out=y[:, :],
                                out_offset=bass.IndirectOffsetOnAxis(ap=tok_i[ct][:, :1], axis=0),
                                in_=ye[:], in_offset=None,
                                element_offset=dd * 512,
                                bounds_check=T - 1, oob_is_err=False,
                                compute_op=(AL.bypass if e == 0 else AL.add))

    nc.finalize()
    return nc


def _prep_inputs(x, Wg, W1, b1, W2, b2):
    x = np.asarray(x, np.float32).reshape(T, D)
    xt = np.ascontiguousarray(x.T)
    xb = np.vstack([x, np.zeros((1, D), np.float32)]).astype(ml_dtypes.bfloat16)
    tokids = (np.arange(NB, dtype=np.float32)[None, :] * 128
              + np.arange(128, dtype=np.float32)[:, None])
    iota16 = np.broadcast_to(np.arange(E, dtype=np.float32), (128, E)).copy()
    onescol = np.ones((128, 1), np.float32)
    tri128 = np.triu(np.ones((128, 128), np.float32))
    identb = np.eye(128, dtype=np.float32).astype(ml_dtypes.bfloat16)
    Wg = np.asarray(Wg, np.float32)
    W1 = np.asarray(W1, np.float32)
    W2 = np.asarray(W2, np.float32)
    b1 = np.asarray(b1, np.float32)
    b2 = np.asarray(b2, np.float32)
    in_maps = []
    for c in range(8):
        el = slice(2 * c, 2 * c + 2)
        w1h = np.ascontiguousarray(
            W1[el].reshape(EL, NK, 128, NF, 128).transpose(0, 3, 2, 1, 4)
        ).astype(ml_dtypes.bfloat16)
        w2h = np.ascontiguousarray(
            W2[el].reshape(EL, NF, 128, ND, 512).transpose(0, 3, 1, 2, 4)
        ).astype(ml_dtypes.bfloat16)
        b1h = np.ascontiguousarray(b1[el].reshape(EL, NF, 128).transpose(0, 2, 1))
        b2d = np.ascontiguousarray(b2[el])
        basev = np.full((128, 1), 1280.0 * c, np.float32)
        in_maps.append(dict(xt=xt, xb=xb, wg=Wg, w1h=w1h, w2h=w2h, b1h=b1h,
                            b2d=b2d, tokids=tokids, iota16=iota16,
                            onescol=onescol, tri128=tri128, identb=identb,
                            basev=basev))
    return in_maps


def _run(inputs, trace=False, trace_cores=None):
    if "nc" not in _CACHE:
        _CACHE["nc"] = _build_nc()
    nc = _CACHE["nc"]
    in_maps = _prep_inputs(inputs["x"], inputs["Wg"], inputs["W1"],
                           inputs["b1"], inputs["W2"], inputs["b2"])
    res = run_bass_kernel_spmd(nc, in_maps, list(range(8)), trace=trace,
                               trace_cores=trace_cores)
    y = np.zeros((T, D), np.float64)
    for r in res.results:
        y += r["y"].astype(np.float64)
    y = y.astype(np.float32).reshape(B, S, D)
    return y, res


def kernel(x, Wg, W1, b1, W2, b2):
    y, _ = _run(dict(x=x, Wg=Wg, W1=W1, b1=b1, W2=W2, b2=b2))
    return y


# revision 5
# speedup vs baseline: 1.0638x; 1.0638x over previous
"""MoE block (B=2,S=2048,D=2048,FF=8192,E=16,K=2,C=640) on 8 trn2 cores.

Expert parallelism: 2 experts per core. Each core redundantly computes the
top-2 gate + capacity positions (exact fp32), dispatches its local experts'
tokens via indirect DMA gather, runs the expert FFN in bf16 (fp32 accum),
and scatter-adds gate-weighted rows into a per-core partial output.
Host sums the 8 partials (expert-parallel combine/unshard).
"""
import sys
sys.path.insert(0, "/opt/trn_rl_repo")
import numpy as np
import ml_dtypes

import concourse.bass as bass
import concourse.mybir as mybir
import concourse.tile as tile
from concourse import bacc
from concourse.bass_utils import run_bass_kernel_spmd

F32 = mybir.dt.float32
BF16 = mybir.dt.bfloat16
I32 = mybir.dt.int32
U32 = mybir.dt.uint32
AL = mybir.AluOpType
ACTF = mybir.ActivationFunctionType

B, S, D, FF, E, K = 2, 2048, 2048, 8192, 16, 2
T = B * S                 # 4096 tokens
C = 640                   # per-expert capacity
NB = T // 128             # 32 token blocks
EL = 2                    # local experts per core
NF = FF // 128            # 64 f-tiles
ND = D // 512             # 4 dd chunks
NCT = C // 128            # 5 capacity tiles per expert
NK = D // 128             # 16 contraction tiles of D
NQ = 4                    # slot-table split (parallel scatter chains)
SENT = float(T)           # sentinel token id -> zero row of xb

_CACHE = {}


def _build_nc():
    nc = bacc.Bacc(None, target_bir_lowering=False, debug=True)

    xt = nc.dram_tensor("xt", [D, T], F32, kind="ExternalInput")
    xb = nc.dram_tensor("xb", [T + 1, D], BF16, kind="ExternalInput")
    wg = nc.dram_tensor("wg", [D, E], F32, kind="ExternalInput")
    w1h = nc.dram_tensor("w1h", [EL, NF, 128, NK, 128], BF16, kind="ExternalInput")
    w2h = nc.dram_tensor("w2h", [EL, ND, NF, 128, 512], BF16, kind="ExternalInput")
    b1h = nc.dram_tensor("b1h", [EL, 128, NF], F32, kind="ExternalInput")
    b2d = nc.dram_tensor("b2d", [EL, D], BF16, kind="ExternalInput")
    tokids = nc.dram_tensor("tokids", [128, NB], F32, kind="ExternalInput")
    iota16 = nc.dram_tensor("iota16", [128, E], F32, kind="ExternalInput")
    onescol = nc.dram_tensor("onescol", [128, 1], F32, kind="ExternalInput")
    tri128 = nc.dram_tensor("tri128", [128, 128], F32, kind="ExternalInput")
    identb = nc.dram_tensor("identb", [128, 128], BF16, kind="ExternalInput")
    basev = nc.dram_tensor("basev", [128, 1], F32, kind="ExternalInput")

    slots = [nc.dram_tensor(f"slots{q}", [EL * C, 2], F32) for q in range(NQ)]
    exclb = nc.dram_tensor("exclb", [2, NB * E], F32)

    y = nc.dram_tensor("y", [T, D], F32, kind="ExternalOutput")

    with tile.TileContext(nc) as tc:
        with tc.tile_pool(name="consts", bufs=1) as cp:
            wg_sb = cp.tile([128, NK * E], F32)
            for k in range(NK):
                nc.sync.dma_start(wg_sb[:, k * E:(k + 1) * E], wg[k * 128:(k + 1) * 128, :])
            iota_sb = cp.tile([128, E], F32)
            nc.sync.dma_start(iota_sb[:], iota16[:])
            ones_sb = cp.tile([128, 1], F32)
            nc.sync.dma_start(ones_sb[:], onescol[:])
            tri_sb = cp.tile([128, 128], F32)
            nc.sync.dma_start(tri_sb[:], tri128[:])
            idb_sb = cp.tile([128, 128], BF16)
            nc.sync.dma_start(idb_sb[:], identb[:])
            base_sb = cp.tile([128, 1], F32)
            nc.sync.dma_start(base_sb[:], basev[:])
            tok_sb = cp.tile([128, NB], F32)
            nc.sync.dma_start(tok_sb[:], tokids[:])
            b1_sb = [cp.tile([128, NF], F32, tag=f"b1_{e}", name=f"b1_{e}")
                     for e in range(EL)]
            b2_sb = [cp.tile([128, D], BF16, tag=f"b2_{e}", name=f"b2_{e}")
                     for e in range(EL)]
            for e in range(EL):
                nc.sync.dma_start(b1_sb[e][:], b1h[e])
                nc.sync.dma_start(b2_sb[e][:], b2d[e:e + 1, :].to_broadcast([128, D]))

            # sentinel-init the slot tables: tok=T (zero row of xb), gate=0
            sent = cp.tile([128, 2], F32)
            nc.vector.memset(sent[:, 0:1], SENT)
            nc.vector.memset(sent[:, 1:2], 0.0)
            for q in range(NQ):
                for n in range(EL * C // 128):
                    nc.sync.dma_start(slots[q][n * 128:(n + 1) * 128, :], sent[:])

            # ---------------- routing (replicated on every core) ----------------
            with tc.tile_pool(name="rout", bufs=1) as rp, \
                 tc.tile_pool(name="psr", bufs=1, space="PSUM") as pr:
                logits = rp.tile([128, NB * E], F32)
                mx_all = rp.tile([128, NB, 8], F32)
                mi_all = rp.tile([128, NB, 8], U32)
                oh0 = rp.tile([128, NB * E], F32)
                oh1 = rp.tile([128, NB * E], F32)

                # gate: logits[t,e] = sum_d xT[d,t]*Wg[d,e]; xT streamed in
                # [128,512] chunks, stationary = xT 128-slices
                for tcb in range(T // 512):
                    xtg = [rp.tile([128, 512], F32, tag=f"xtg{k}", name=f"xtg{k}",
                                   bufs=2) for k in range(NK)]
                    for k in range(NK):
                        nc.sync.dma_start(
                            xtg[k][:], xt[k * 128:(k + 1) * 128,
                                          tcb * 512:(tcb + 1) * 512])
                    for j in range(4):
                        b = tcb * 4 + j
                        lg = pr.tile([128, E], F32, tag="lg", bufs=2)
                        for k in range(NK):
                            nc.tensor.matmul(lg[:], lhsT=xtg[k][:, j * 128:(j + 1) * 128],
                                             rhs=wg_sb[:, k * E:(k + 1) * E],
                                             start=(k == 0), stop=(k == NK - 1))
                        sl = slice(b * E, (b + 1) * E)
                        nc.vector.tensor_copy(logits[:, sl], lg[:])
                        nc.vector.max(out=mx_all[:, b, :], in_=logits[:, sl])
                        nc.vector.max_index(out=mi_all[:, b, :], in_max=mx_all[:, b, :],
                                            in_values=logits[:, sl])

                i0f = rp.tile([128, NB], F32)
                i1f = rp.tile([128, NB], F32)
                nc.vector.tensor_copy(i0f[:], mi_all[:, :, 0])
                nc.vector.tensor_copy(i1f[:], mi_all[:, :, 1])
                for b in range(NB):
                    sl = slice(b * E, (b + 1) * E)
                    nc.vector.tensor_tensor(out=oh0[:, sl], in0=iota_sb[:],
                                            in1=i0f[:, b:b + 1].to_broadcast([128, E]),
                                            op=AL.is_equal)
                    nc.vector.tensor_tensor(out=oh1[:, sl], in0=iota_sb[:],
                                            in1=i1f[:, b:b + 1].to_broadcast([128, E]),
                                            op=AL.is_equal)

                # gates from top-2 logits: g0 = 1/(1+e^(l1-l0)), g1 = 1-g0
                l0 = mx_all[:, :, 0]
                l1 = mx_all[:, :, 1]
                dte = rp.tile([128, NB], F32)
                nc.vector.tensor_tensor(out=dte[:], in0=l1, in1=l0, op=AL.subtract)
                exd = rp.tile([128, NB], F32)
                nc.scalar.activation(exd[:], dte[:], ACTF.Exp)
                den = rp.tile([128, NB], F32)
                nc.vector.tensor_scalar_add(den[:], exd[:], 1.0)
                g0 = rp.tile([128, NB], F32)
                nc.vector.reciprocal(g0[:], den[:])
                g1 = rp.tile([128, NB], F32)
                nc.vector.tensor_tensor(out=g1[:], in0=exd[:], in1=g0[:], op=AL.mult)

                # per-expert counts per block: countsT[e, b] via ones-matmuls
                cnt_ps = [pr.tile([E, NB], F32, tag=f"cnt{s}", name=f"cnt{s}", bufs=1)
                          for s in range(2)]
                for b in range(NB):
                    sl = slice(b * E, (b + 1) * E)
                    nc.tensor.matmul(cnt_ps[0][:, b:b + 1], lhsT=oh0[:, sl], rhs=ones_sb[:])
                    nc.tensor.matmul(cnt_ps[1][:, b:b + 1], lhsT=oh1[:, sl], rhs=ones_sb[:])
                zer = rp.tile([E, NB], F32)
                nc.vector.memset(zer[:], 0.0)
                incl = [rp.tile([E, NB], F32, tag=f"incl{s}", name=f"incl{s}")
                        for s in range(2)]
                excl = [rp.tile([E, NB], F32, tag=f"excl{s}", name=f"excl{s}")
                        for s in range(2)]
                for s in range(2):
                    nc.vector.tensor_tensor_scan(out=incl[s][:], data0=cnt_ps[s][:],
                                                 data1=zer[:], initial=0.0,
                                                 op0=AL.add, op1=AL.add)
                    nc.vector.tensor_tensor(out=excl[s][:], in0=incl[s][:],
                                            in1=cnt_ps[s][:], op=AL.subtract)
                # slot-1 block offsets additionally shifted by total slot-0 counts
                nc.vector.tensor_tensor(out=excl[1][:], in0=excl[1][:],
                                        in1=incl[0][:, NB - 1:NB].to_broadcast([E, NB]),
                                        op=AL.add)
                for s in range(2):
                    nc.sync.dma_start(
                        exclb[s:s + 1, :].rearrange("o (b e) -> (o e) b", e=E),
                        excl[s][:])
                bc = [rp.tile([128, NB * E], F32, tag=f"bc{s}", name=f"bc{s}")
                      for s in range(2)]
                for s in range(2):
                    nc.sync.dma_start(bc[s][:],
                                      exclb[s:s + 1, :].to_broadcast([128, NB * E]))

                # positions: in-block inclusive cumsum (triangular matmul) + block offset
                pos = [rp.tile([128, NB], F32, tag=f"pos{s}", name=f"pos{s}")
                       for s in range(2)]
                for s, oh in enumerate((oh0, oh1)):
                    for b in range(NB):
                        sl = slice(b * E, (b + 1) * E)
                        cu = pr.tile([128, E], F32, tag="cum", bufs=2)
                        nc.tensor.matmul(cu[:], lhsT=tri_sb[:], rhs=oh[:, sl])
                        t1 = rp.tile([128, E], F32, tag="pt", bufs=3)
                        nc.vector.tensor_tensor(out=t1[:], in0=cu[:], in1=bc[s][:, sl],
                                                op=AL.add)
                        nc.vector.tensor_tensor(out=t1[:], in0=t1[:], in1=oh[:, sl],
                                                op=AL.mult)
                        nc.vector.tensor_reduce(out=pos[s][:, b:b + 1], in_=t1[:],
                                                axis=mybir.AxisListType.X, op=AL.add)
                    nc.vector.tensor_scalar_add(pos[s][:], pos[s][:], -1.0)

                # scatter (token_id, gate) into the 4 local slot tables
                for s, (idxf, gv) in enumerate(((i0f, g0), (i1f, g1))):
                    offc = rp.tile([128, NB], F32, tag=f"off{s}", name=f"off{s}")
                    nc.vector.tensor_scalar_mul(offc[:], idxf[:], float(C))
                    nc.vector.tensor_tensor(out=offc[:], in0=offc[:], in1=pos[s][:],
                                            op=AL.add)
                    nc.vector.tensor_scalar_sub(offc[:], offc[:], base_sb[:, 0:1])
                    m1 = rp.tile([128, NB], F32, tag="m1s")
                    m2 = rp.tile([128, NB], F32, tag="m2s")
                    nc.vector.tensor_scalar(m1[:], offc[:], 0.0, None, op0=AL.is_ge)
                    nc.vector.tensor_scalar(m2[:], offc[:], float(EL * C), None,
                                            op0=AL.is_lt)
                    nc.vector.tensor_tensor(out=m1[:], in0=m1[:], in1=m2[:], op=AL.mult)
                    nc.vector.tensor_scalar(m2[:], pos[s][:], float(C), None,
                                            op0=AL.is_lt)
                    nc.vector.tensor_tensor(out=m1[:], in0=m1[:], in1=m2[:], op=AL.mult)
                    nc.vector.tensor_tensor(out=offc[:], in0=offc[:], in1=m1[:],
                                            op=AL.mult)
                    nc.vector.tensor_scalar(m2[:], m1[:], -2.0e9, 2.0e9,
                                            op0=AL.mult, op1=AL.add)
                    nc.vector.tensor_tensor(out=offc[:], in0=offc[:], in1=m2[:],
                                            op=AL.add)
                    offi = rp.tile([128, NB], I32, tag=f"offi{s}", name=f"offi{s}")
                    nc.vector.tensor_copy(offi[:], offc[:])
                    pay = rp.tile([128, NB, 2], F32, tag=f"pay{s}", name=f"pay{s}")
                    nc.vector.tensor_copy(pay[:, :, 0], tok_sb[:])
                    nc.vector.tensor_copy(pay[:, :, 1], gv[:])
                    for b in range(NB):
                        q = b % NQ
                        nc.gpsimd.indirect_dma_start(
                            out=slots[q][:, :],
                            out_offset=bass.IndirectOffsetOnAxis(ap=offi[:, b:b + 1],
                                                                 axis=0),
                            in_=pay[:, b, :], in_offset=None,
                            bounds_check=EL * C - 1, oob_is_err=False)

            # ---------------- expert FFN + combine ----------------
            with tc.tile_pool(name="ffn", bufs=1) as fp, \
                 tc.tile_pool(name="psf", bufs=1, space="PSUM") as pf:
                for e in range(EL):
                    slot_sb = [fp.tile([128, 2], F32, tag=f"slot{ct}", name=f"slot{ct}",
                                       bufs=2) for ct in range(NCT)]
                    tok_i = [fp.tile([128, 1], I32, tag=f"toki{ct}", name=f"toki{ct}",
                                     bufs=2) for ct in range(NCT)]
                    xte = [fp.tile([128, C], BF16, tag=f"xte{k}", name=f"xte{k}",
                                   bufs=1) for k in range(NK)]
                    for ct in range(NCT):
                        r0 = e * C + ct * 128
                        # merge the 4 slot tables (disjoint coverage, sentinel
                        # everywhere else)
                        nc.sync.dma_start(slot_sb[ct][:], slots[0][r0:r0 + 128, :])
                        for q in range(1, NQ):
                            sq = fp.tile([128, 2], F32, tag="sq", bufs=3)
                            nc.sync.dma_start(sq[:], slots[q][r0:r0 + 128, :])
                            msk = fp.tile([128, 1], I32, tag="msk", bufs=3)
                            nc.vector.tensor_scalar(msk[:], sq[:, 0:1], SENT, None,
                                                    op0=AL.is_lt)
                            nc.vector.copy_predicated(
                                out=slot_sb[ct][:], mask=msk[:].to_broadcast([128, 2]),
                                data=sq[:])
                        nc.vector.tensor_copy(tok_i[ct][:], slot_sb[ct][:, 0:1])
                        xg = fp.tile([128, D], BF16, tag="xg", bufs=2)
                        nc.gpsimd.indirect_dma_start(
                            out=xg[:], out_offset=None, in_=xb[:, :],
                            in_offset=bass.IndirectOffsetOnAxis(ap=tok_i[ct][:, :1],
                                                                axis=0))
                        for k in range(NK):
                            tp = pf.tile([128, 128], BF16, tag="ptr", bufs=1)
                            nc.tensor.transpose(out=tp[:],
                                                in_=xg[:, k * 128:(k + 1) * 128],
                                                identity=idb_sb[:])
                            nc.vector.tensor_copy(xte[k][:, ct * 128:(ct + 1) * 128],
                                                  tp[:])

                    # mm1 + GELU: hT[f] = gelu(W1[:,f].T @ X.T + b1[f])
                    ht = [fp.tile([128, C], BF16, tag=f"ht{f}", name=f"ht{f}", bufs=1)
                          for f in range(NF)]
                    for f in range(NF):
                        w1c = fp.tile([128, NK * 128], BF16, tag="w1c", bufs=3)
                        nc.sync.dma_start(w1c[:], w1h[e, f])
                        psA = pf.tile([128, 320], F32, tag="m1", bufs=2)
                        psB = pf.tile([128, 320], F32, tag="m1", bufs=2)
                        for k in range(NK):
                            lw = w1c[:, k * 128:(k + 1) * 128]
                            nc.tensor.matmul(psA[:], lhsT=lw, rhs=xte[k][:, 0:320],
                                             start=(k == 0), stop=(k == NK - 1))
                            nc.tensor.matmul(psB[:], lhsT=lw, rhs=xte[k][:, 320:640],
                                             start=(k == 0), stop=(k == NK - 1))
                        nc.scalar.activation(ht[f][:, 0:320], psA[:], ACTF.Gelu,
                                             bias=b1_sb[e][:, f:f + 1])
                        nc.scalar.activation(ht[f][:, 320:640], psB[:], ACTF.Gelu,
                                             bias=b1_sb[e][:, f:f + 1])

                    # mm2 + bias + gate-scale; scatter one full row-tile per ct
                    yrow = [fp.tile([128, D], F32, tag=f"yrow{ct}", name=f"yrow{ct}",
                                    bufs=1) for ct in range(NCT)]
                    for dd in range(ND):
                        psY = [pf.tile([128, 512], F32, tag=f"m2_{ct}", name=f"m2_{ct}",
                                       bufs=1) for ct in range(NCT)]
                        for f in range(NF):
                            w2c = fp.tile([128, 512], BF16, tag="w2c", bufs=3)
                            nc.sync.dma_start(w2c[:], w2h[e, dd, f])
                            for ct in range(NCT):
                                nc.tensor.matmul(psY[ct][:],
                                                 lhsT=ht[f][:, ct * 128:(ct + 1) * 128],
                                                 rhs=w2c[:],
                                                 start=(f == 0), stop=(f == NF - 1))
                        for ct in range(NCT):
                            dsl = slice(dd * 512, (dd + 1) * 512)
                            nc.vector.tensor_tensor(out=yrow[ct][:, dsl], in0=psY[ct][:],
                                                    in1=b2_sb[e][:, dsl], op=AL.add)
                            nc.vector.tensor_scalar_mul(yrow[ct][:, dsl],
                                                        yrow[ct][:, dsl],
                                                        slot_sb[ct][:, 1:2])
                    for ct in range(NCT):
                        nc.gpsimd.indirect_dma_start(
                            out=y[:, :],
                            out_offset=bass.IndirectOffsetOnAxis(ap=tok_i[ct][:, :1],
                                                                 axis=0),
                            in_=yrow[ct][:], in_offset=None,
                            bounds_check=T - 1, oob_is_err=False,
                            compute_op=(AL.bypass if e == 0 else AL.add))

    nc.finalize()
    return nc


def _prep_inputs(x, Wg, W1, b1, W2, b2):
    x = np.asarray(x, np.float32).reshape(T, D)
    xt = np.ascontiguousarray(x.T)
    xb = np.vstack([x, np.zeros((1, D), np.float32)]).astype(ml_dtypes.bfloat16)
    tokids = (np.arange(NB, dtype=np.float32)[None, :] * 128
              + np.arange(128, dtype=np.float32)[:, None])
    iota16 = np.broadcast_to(np.arange(E, dtype=np.float32), (128, E)).copy()
    onescol = np.ones((128, 1), np.float32)
    tri128 = np.triu(np.ones((128, 128), np.float32))
    identb = np.eye(128, dtype=np.float32).astype(ml_dtypes.bfloat16)
    Wg = np.asarray(Wg, np.float32)
    W1 = np.asarray(W1, np.float32)
    W2 = np.asarray(W2, np.float32)
    b1 = np.asarray(b1, np.float32)
    b2 = np.asarray(b2, np.float32)
    in_maps = []
    for c in range(8):
        el = slice(2 * c, 2 * c + 2)
        w1h = np.ascontiguousarray(
            W1[el].reshape(EL, NK, 128, NF, 128).transpose(0, 3, 2, 1, 4)
        ).astype(ml_dtypes.bfloat16)
        w2h = np.ascontiguousarray(
            W2[el].reshape(EL, NF, 128, ND, 512).transpose(0, 3, 1, 2, 4)
        ).astype(ml_dtypes.bfloat16)
        b1h = np.ascontiguousarray(b1[el].reshape(EL, NF, 128).transpose(0, 2, 1))
        b2d = np.ascontiguousarray(b2[el]).astype(ml_dtypes.bfloat16)
        basev = np.full((128, 1), 1280.0 * c, np.float32)
        in_maps.append(dict(xt=xt, xb=xb, wg=Wg, w1h=w1h, w2h=w2h, b1h=b1h,
                            b2d=b2d, tokids=tokids, iota16=iota16,
                            onescol=onescol, tri128=tri128, identb=identb,
                            basev=basev))
    return in_maps


def _run(inputs, trace=False, trace_cores=None):
    if "nc" not in _CACHE:
        _CACHE["nc"] = _build_nc()
    nc = _CACHE["nc"]
    in_maps = _prep_inputs(inputs["x"], inputs["Wg"], inputs["W1"],
                           inputs["b1"], inputs["W2"], inputs["b2"])
    res = run_bass_kernel_spmd(nc, in_maps, list(range(8)), trace=trace,
                               trace_cores=trace_cores)
    y = np.zeros((T, D), np.float64)
    for r in res.results:
        y += r["y"].astype(np.float64)
    y = y.astype(np.float32).reshape(B, S, D)
    return y, res


def kernel(x, Wg, W1, b1, W2, b2):
    y, _ = _run(dict(x=x, Wg=Wg, W1=W1, b1=b1, W2=W2, b2=b2))
    return y


# revision 7
# speedup vs baseline: 1.1342x; 1.0662x over previous
"""MoE block (B=2,S=2048,D=2048,FF=8192,E=16,K=2,C=640) on 8 trn2 cores.

Expert parallelism: 2 experts per core. Each core redundantly computes the
top-2 gate + capacity positions (exact fp32), dispatches its local experts'
tokens via indirect DMA gather, runs the expert FFN in bf16 (fp32 accum),
and scatter-adds gate-weighted rows into a per-core partial output.
Host sums the 8 partials (expert-parallel combine/unshard).
"""
import sys
sys.path.insert(0, "/opt/trn_rl_repo")
import numpy as np
import ml_dtypes

import concourse.bass as bass
import concourse.mybir as mybir
import concourse.tile as tile
from concourse import bacc
from concourse.bass_utils import run_bass_kernel_spmd

F32 = mybir.dt.float32
BF16 = mybir.dt.bfloat16
I32 = mybir.dt.int32
U32 = mybir.dt.uint32
AL = mybir.AluOpType
ACTF = mybir.ActivationFunctionType

B, S, D, FF, E, K = 2, 2048, 2048, 8192, 16, 2
T = B * S                 # 4096 tokens
C = 640                   # per-expert capacity
NB = T // 128             # 32 token blocks
EL = 2                    # local experts per core
NF = FF // 128            # 64 f-tiles
ND = D // 512             # 4 dd chunks
NCT = C // 128            # 5 capacity tiles per expert
NK = D // 128             # 16 contraction tiles of D
NQ = 4                    # slot-table split (parallel scatter chains)
SENT = float(T)           # sentinel token id -> zero row of xb

_CACHE = {}


def _build_nc():
    nc = bacc.Bacc(None, target_bir_lowering=False, debug=True)

    xtloc = nc.dram_tensor("xtloc", [D, 512], F32, kind="ExternalInput")
    xb = nc.dram_tensor("xb", [T + 1, D], BF16, kind="ExternalInput")
    wg = nc.dram_tensor("wg", [D, E], F32, kind="ExternalInput")
    w1h = nc.dram_tensor("w1h", [EL, NF, 128, NK, 128], BF16, kind="ExternalInput")
    w2h = nc.dram_tensor("w2h", [EL, ND, NF, 128, 512], BF16, kind="ExternalInput")
    b1h = nc.dram_tensor("b1h", [EL, 128, NF], F32, kind="ExternalInput")
    b2d = nc.dram_tensor("b2d", [EL, D], BF16, kind="ExternalInput")
    tokids = nc.dram_tensor("tokids", [128, NB], F32, kind="ExternalInput")
    iota16 = nc.dram_tensor("iota16", [128, E], F32, kind="ExternalInput")
    onescol = nc.dram_tensor("onescol", [128, 1], F32, kind="ExternalInput")
    tri128 = nc.dram_tensor("tri128", [128, 128], F32, kind="ExternalInput")
    identb = nc.dram_tensor("identb", [128, 128], BF16, kind="ExternalInput")
    basev = nc.dram_tensor("basev", [128, 1], F32, kind="ExternalInput")

    slots = [nc.dram_tensor(f"slots{q}", [EL * C, 2], F32) for q in range(NQ)]
    exclb = nc.dram_tensor("exclb", [2, NB * E], F32)
    loclg = nc.dram_tensor("loclg", [512, E], F32)
    gathlg = nc.dram_tensor("gathlg", [T, E], F32, addr_space="Shared")

    y = nc.dram_tensor("y", [T, D], F32, kind="ExternalOutput")

    with tile.TileContext(nc) as tc:
        with tc.tile_pool(name="consts", bufs=1) as cp:
            wg_sb = cp.tile([128, NK * E], F32)
            for k in range(NK):
                nc.sync.dma_start(wg_sb[:, k * E:(k + 1) * E], wg[k * 128:(k + 1) * 128, :])
            iota_sb = cp.tile([128, E], F32)
            nc.sync.dma_start(iota_sb[:], iota16[:])
            ones_sb = cp.tile([128, 1], F32)
            nc.sync.dma_start(ones_sb[:], onescol[:])
            tri_sb = cp.tile([128, 128], F32)
            nc.sync.dma_start(tri_sb[:], tri128[:])
            idb_sb = cp.tile([128, 128], BF16)
            nc.sync.dma_start(idb_sb[:], identb[:])
            base_sb = cp.tile([128, 1], F32)
            nc.sync.dma_start(base_sb[:], basev[:])
            tok_sb = cp.tile([128, NB], F32)
            nc.sync.dma_start(tok_sb[:], tokids[:])
            b1_sb = [cp.tile([128, NF], F32, tag=f"b1_{e}", name=f"b1_{e}")
                     for e in range(EL)]
            b2_sb = [cp.tile([128, D], BF16, tag=f"b2_{e}", name=f"b2_{e}")
                     for e in range(EL)]
            for e in range(EL):
                nc.sync.dma_start(b1_sb[e][:], b1h[e])
                nc.sync.dma_start(b2_sb[e][:], b2d[e:e + 1, :].to_broadcast([128, D]))

            # sentinel-init the slot tables: tok=T (zero row of xb), gate=0
            sent = cp.tile([128, 2], F32)
            nc.vector.memset(sent[:, 0:1], SENT)
            nc.vector.memset(sent[:, 1:2], 0.0)
            for q in range(NQ):
                for n in range(EL * C // 128):
                    nc.sync.dma_start(slots[q][n * 128:(n + 1) * 128, :], sent[:])

            # ---------------- routing (replicated on every core) ----------------
            with tc.tile_pool(name="rout", bufs=1) as rp, \
                 tc.tile_pool(name="psr", bufs=1, space="PSUM") as pr:
                logits = rp.tile([128, NB * E], F32)
                mx_all = rp.tile([128, NB, 8], F32)
                mi_all = rp.tile([128, NB, 8], U32)
                oh0 = rp.tile([128, NB * E], F32)
                oh1 = rp.tile([128, NB * E], F32)

                # gate (data-parallel): this core computes logits for its
                # 512-token stripe, then AllGather replicates [T, E]
                xtg = [rp.tile([128, 512], F32, tag=f"xtg{k}", name=f"xtg{k}",
                               bufs=1) for k in range(NK)]
                for k in range(NK):
                    nc.sync.dma_start(xtg[k][:], xtloc[k * 128:(k + 1) * 128, :])
                locsb = rp.tile([128, 4, E], F32)
                for j in range(4):
                    lg = pr.tile([128, E], F32, tag="lg", bufs=2)
                    for k in range(NK):
                        nc.tensor.matmul(lg[:], lhsT=xtg[k][:, j * 128:(j + 1) * 128],
                                         rhs=wg_sb[:, k * E:(k + 1) * E],
                                         start=(k == 0), stop=(k == NK - 1))
                    nc.vector.tensor_copy(locsb[:, j, :], lg[:])
                nc.sync.dma_start(loclg[:].rearrange("(n p) e -> p n e", p=128),
                                  locsb[:])
                nc.gpsimd.collective_compute(
                    "AllGather", AL.bypass, replica_groups=[list(range(8))],
                    ins=[loclg[:]], outs=[gathlg[:]])
                for tcb in range(T // 512):
                    lgt = rp.tile([128, 4, E], F32, tag="lgt", bufs=4)
                    nc.sync.dma_start(
                        lgt[:], gathlg[tcb * 512:(tcb + 1) * 512, :].rearrange(
                            "(n p) e -> p n e", p=128))
                    for j in range(4):
                        b = tcb * 4 + j
                        sl = slice(b * E, (b + 1) * E)
                        nc.vector.tensor_copy(logits[:, sl], lgt[:, j, :])
                        nc.vector.max(out=mx_all[:, b, :], in_=logits[:, sl])
                        nc.vector.max_index(out=mi_all[:, b, :], in_max=mx_all[:, b, :],
                                            in_values=logits[:, sl])

                i0f = rp.tile([128, NB], F32)
                i1f = rp.tile([128, NB], F32)
                nc.vector.tensor_copy(i0f[:], mi_all[:, :, 0])
                nc.vector.tensor_copy(i1f[:], mi_all[:, :, 1])
                for b in range(NB):
                    sl = slice(b * E, (b + 1) * E)
                    nc.vector.tensor_tensor(out=oh0[:, sl], in0=iota_sb[:],
                                            in1=i0f[:, b:b + 1].to_broadcast([128, E]),
                                            op=AL.is_equal)
                    nc.vector.tensor_tensor(out=oh1[:, sl], in0=iota_sb[:],
                                            in1=i1f[:, b:b + 1].to_broadcast([128, E]),
                                            op=AL.is_equal)

                # gates from top-2 logits: g0 = 1/(1+e^(l1-l0)), g1 = 1-g0
                l0 = mx_all[:, :, 0]
                l1 = mx_all[:, :, 1]
                dte = rp.tile([128, NB], F32)
                nc.vector.tensor_tensor(out=dte[:], in0=l1, in1=l0, op=AL.subtract)
                exd = rp.tile([128, NB], F32)
                nc.scalar.activation(exd[:], dte[:], ACTF.Exp)
                den = rp.tile([128, NB], F32)
                nc.vector.tensor_scalar_add(den[:], exd[:], 1.0)
                g0 = rp.tile([128, NB], F32)
                nc.vector.reciprocal(g0[:], den[:])
                g1 = rp.tile([128, NB], F32)
                nc.vector.tensor_tensor(out=g1[:], in0=exd[:], in1=g0[:], op=AL.mult)

                # per-expert counts per block: countsT[e, b] via ones-matmuls
                cnt_ps = [pr.tile([E, NB], F32, tag=f"cnt{s}", name=f"cnt{s}", bufs=1)
                          for s in range(2)]
                for b in range(NB):
                    sl = slice(b * E, (b + 1) * E)
                    nc.tensor.matmul(cnt_ps[0][:, b:b + 1], lhsT=oh0[:, sl], rhs=ones_sb[:])
                    nc.tensor.matmul(cnt_ps[1][:, b:b + 1], lhsT=oh1[:, sl], rhs=ones_sb[:])
                zer = rp.tile([E, NB], F32)
                nc.vector.memset(zer[:], 0.0)
                incl = [rp.tile([E, NB], F32, tag=f"incl{s}", name=f"incl{s}")
                        for s in range(2)]
                excl = [rp.tile([E, NB], F32, tag=f"excl{s}", name=f"excl{s}")
                        for s in range(2)]
                for s in range(2):
                    nc.vector.tensor_tensor_scan(out=incl[s][:], data0=cnt_ps[s][:],
                                                 data1=zer[:], initial=0.0,
                                                 op0=AL.add, op1=AL.add)
                    nc.vector.tensor_tensor(out=excl[s][:], in0=incl[s][:],
                                            in1=cnt_ps[s][:], op=AL.subtract)
                # slot-1 block offsets additionally shifted by total slot-0 counts
                nc.vector.tensor_tensor(out=excl[1][:], in0=excl[1][:],
                                        in1=incl[0][:, NB - 1:NB].to_broadcast([E, NB]),
                                        op=AL.add)
                for s in range(2):
                    nc.sync.dma_start(
                        exclb[s:s + 1, :].rearrange("o (b e) -> (o e) b", e=E),
                        excl[s][:])
                bc = [rp.tile([128, NB * E], F32, tag=f"bc{s}", name=f"bc{s}")
                      for s in range(2)]
                for s in range(2):
                    nc.sync.dma_start(bc[s][:],
                                      exclb[s:s + 1, :].to_broadcast([128, NB * E]))

                # positions: in-block inclusive cumsum (triangular matmul) + block offset
                pos = [rp.tile([128, NB], F32, tag=f"pos{s}", name=f"pos{s}")
                       for s in range(2)]
                for s, (oh, idxf, gv) in enumerate(((oh0, i0f, g0), (oh1, i1f, g1))):
                    for b in range(NB):
                        sl = slice(b * E, (b + 1) * E)
                        cu = pr.tile([128, E], F32, tag="cum", bufs=2)
                        nc.tensor.matmul(cu[:], lhsT=tri_sb[:], rhs=oh[:, sl])
                        t1 = rp.tile([128, E], F32, tag="pt", bufs=3)
                        nc.vector.tensor_tensor(out=t1[:], in0=cu[:], in1=bc[s][:, sl],
                                                op=AL.add)
                        nc.vector.tensor_tensor(out=t1[:], in0=t1[:], in1=oh[:, sl],
                                                op=AL.mult)
                        nc.vector.tensor_reduce(out=pos[s][:, b:b + 1], in_=t1[:],
                                                axis=mybir.AxisListType.X, op=AL.add)
                    nc.vector.tensor_scalar_add(pos[s][:], pos[s][:], -1.0)
                    offc = rp.tile([128, NB], F32, tag=f"off{s}", name=f"off{s}")
                    nc.vector.tensor_scalar_mul(offc[:], idxf[:], float(C))
                    nc.vector.tensor_tensor(out=offc[:], in0=offc[:], in1=pos[s][:],
                                            op=AL.add)
                    nc.vector.tensor_scalar_sub(offc[:], offc[:], base_sb[:, 0:1])
                    m1 = rp.tile([128, NB], F32, tag="m1s")
                    m2 = rp.tile([128, NB], F32, tag="m2s")
                    nc.vector.tensor_scalar(m1[:], offc[:], 0.0, None, op0=AL.is_ge)
                    nc.vector.tensor_scalar(m2[:], offc[:], float(EL * C), None,
                                            op0=AL.is_lt)
                    nc.vector.tensor_tensor(out=m1[:], in0=m1[:], in1=m2[:], op=AL.mult)
                    nc.vector.tensor_scalar(m2[:], pos[s][:], float(C), None,
                                            op0=AL.is_lt)
                    nc.vector.tensor_tensor(out=m1[:], in0=m1[:], in1=m2[:], op=AL.mult)
                    nc.vector.tensor_tensor(out=offc[:], in0=offc[:], in1=m1[:],
                                            op=AL.mult)
                    nc.vector.tensor_scalar(m2[:], m1[:], -2.0e9, 2.0e9,
                                            op0=AL.mult, op1=AL.add)
                    nc.vector.tensor_tensor(out=offc[:], in0=offc[:], in1=m2[:],
                                            op=AL.add)
                    offi = rp.tile([128, NB], I32, tag=f"offi{s}", name=f"offi{s}")
                    nc.vector.tensor_copy(offi[:], offc[:])
                    pay = rp.tile([128, NB, 2], F32, tag=f"pay{s}", name=f"pay{s}")
                    nc.vector.tensor_copy(pay[:, :, 0], tok_sb[:])
                    nc.vector.tensor_copy(pay[:, :, 1], gv[:])
                    for b in range(NB):
                        q = b % NQ
                        nc.gpsimd.indirect_dma_start(
                            out=slots[q][:, :],
                            out_offset=bass.IndirectOffsetOnAxis(ap=offi[:, b:b + 1],
                                                                 axis=0),
                            in_=pay[:, b, :], in_offset=None,
                            bounds_check=EL * C - 1, oob_is_err=False)

            # ---------------- expert FFN + combine ----------------
            with tc.tile_pool(name="ffn", bufs=1) as fp, \
                 tc.tile_pool(name="psf", bufs=1, space="PSUM") as pf:
                for e in range(EL):
                    slot_sb = [fp.tile([128, 2], F32, tag=f"slot{ct}", name=f"slot{ct}",
                                       bufs=2) for ct in range(NCT)]
                    tok_i = [fp.tile([128, 1], I32, tag=f"toki{ct}", name=f"toki{ct}",
                                     bufs=2) for ct in range(NCT)]
                    xte = [fp.tile([128, C], BF16, tag=f"xte{k}", name=f"xte{k}",
                                   bufs=1) for k in range(NK)]
                    for ct in range(NCT):
                        r0 = e * C + ct * 128
                        # merge the 4 slot tables (disjoint coverage, sentinel
                        # everywhere else)
                        nc.sync.dma_start(slot_sb[ct][:], slots[0][r0:r0 + 128, :])
                        for q in range(1, NQ):
                            sq = fp.tile([128, 2], F32, tag="sq", bufs=3)
                            nc.sync.dma_start(sq[:], slots[q][r0:r0 + 128, :])
                            msk = fp.tile([128, 1], I32, tag="msk", bufs=3)
                            nc.vector.tensor_scalar(msk[:], sq[:, 0:1], SENT, None,
                                                    op0=AL.is_lt)
                            nc.vector.copy_predicated(
                                out=slot_sb[ct][:], mask=msk[:].to_broadcast([128, 2]),
                                data=sq[:])
                        nc.vector.tensor_copy(tok_i[ct][:], slot_sb[ct][:, 0:1])
                        xg = fp.tile([128, D], BF16, tag="xg", bufs=3)
                        nc.gpsimd.indirect_dma_start(
                            out=xg[:], out_offset=None, in_=xb[:, :],
                            in_offset=bass.IndirectOffsetOnAxis(ap=tok_i[ct][:, :1],
                                                                axis=0))
                        for k in range(NK):
                            tp = pf.tile([128, 128], BF16, tag="ptr", bufs=1)
                            nc.tensor.transpose(out=tp[:],
                                                in_=xg[:, k * 128:(k + 1) * 128],
                                                identity=idb_sb[:])
                            nc.vector.tensor_copy(xte[k][:, ct * 128:(ct + 1) * 128],
                                                  tp[:])

                    # mm1 + GELU: hT[f] = gelu(W1[:,f].T @ X.T + b1[f])
                    ht = [fp.tile([128, C], BF16, tag=f"ht{f}", name=f"ht{f}", bufs=1)
                          for f in range(NF)]
                    for f in range(NF):
                        w1c = fp.tile([128, NK * 128], BF16, tag="w1c", bufs=4)
                        nc.sync.dma_start(w1c[:], w1h[e, f])
                        psA = pf.tile([128, 320], F32, tag="m1", bufs=2)
                        psB = pf.tile([128, 320], F32, tag="m1", bufs=2)
                        for k in range(NK):
                            lw = w1c[:, k * 128:(k + 1) * 128]
                            nc.tensor.matmul(psA[:], lhsT=lw, rhs=xte[k][:, 0:320],
                                             start=(k == 0), stop=(k == NK - 1))
                            nc.tensor.matmul(psB[:], lhsT=lw, rhs=xte[k][:, 320:640],
                                             start=(k == 0), stop=(k == NK - 1))
                        nc.scalar.activation(ht[f][:, 0:320], psA[:], ACTF.Gelu,
                                             bias=b1_sb[e][:, f:f + 1])
                        nc.scalar.activation(ht[f][:, 320:640], psB[:], ACTF.Gelu,
                                             bias=b1_sb[e][:, f:f + 1])

                    # mm2 + bias + gate-scale; scatter one full row-tile per ct
                    yrow = [fp.tile([128, D], F32, tag=f"yrow{ct}", name=f"yrow{ct}",
                                    bufs=1) for ct in range(NCT)]
                    for dd in range(ND):
                        psY = [pf.tile([128, 512], F32, tag=f"m2_{ct}", name=f"m2_{ct}",
                                       bufs=1) for ct in range(NCT)]
                        for f in range(NF):
                            w2c = fp.tile([128, 512], BF16, tag="w2c", bufs=6)
                            nc.sync.dma_start(w2c[:], w2h[e, dd, f])
                            for ct in range(NCT):
                                nc.tensor.matmul(psY[ct][:],
                                                 lhsT=ht[f][:, ct * 128:(ct + 1) * 128],
                                                 rhs=w2c[:],
                                                 start=(f == 0), stop=(f == NF - 1))
                        for ct in range(NCT):
                            dsl = slice(dd * 512, (dd + 1) * 512)
                            nc.vector.tensor_tensor(out=yrow[ct][:, dsl], in0=psY[ct][:],
                                                    in1=b2_sb[e][:, dsl], op=AL.add)
                            nc.vector.tensor_scalar_mul(yrow[ct][:, dsl],
                                                        yrow[ct][:, dsl],
                                                        slot_sb[ct][:, 1:2])
                            if dd in (1, ND - 1):
                                h0 = 0 if dd == 1 else 1024
                                nc.gpsimd.indirect_dma_start(
                                    out=y[:, :],
                                    out_offset=bass.IndirectOffsetOnAxis(
                                        ap=tok_i[ct][:, :1], axis=0),
                                    in_=yrow[ct][:, h0:h0 + 1024], in_offset=None,
                                    element_offset=h0,
                                    bounds_check=T - 1, oob_is_err=False,
                                    compute_op=(AL.bypass if e == 0 else AL.add))


    nc.finalize()
    return nc


def _prep_inputs(x, Wg, W1, b1, W2, b2):
    x = np.asarray(x, np.float32).reshape(T, D)
    xtf = np.asarray(x.T, np.float32)
    xb = np.vstack([x, np.zeros((1, D), np.float32)]).astype(ml_dtypes.bfloat16)
    tokids = (np.arange(NB, dtype=np.float32)[None, :] * 128
              + np.arange(128, dtype=np.float32)[:, None])
    iota16 = np.broadcast_to(np.arange(E, dtype=np.float32), (128, E)).copy()
    onescol = np.ones((128, 1), np.float32)
    tri128 = np.triu(np.ones((128, 128), np.float32))
    identb = np.eye(128, dtype=np.float32).astype(ml_dtypes.bfloat16)
    Wg = np.asarray(Wg, np.float32)
    W1 = np.asarray(W1, np.float32)
    W2 = np.asarray(W2, np.float32)
    b1 = np.asarray(b1, np.float32)
    b2 = np.asarray(b2, np.float32)
    in_maps = []
    for c in range(8):
        el = slice(2 * c, 2 * c + 2)
        w1h = np.ascontiguousarray(
            W1[el].reshape(EL, NK, 128, NF, 128).transpose(0, 3, 2, 1, 4)
        ).astype(ml_dtypes.bfloat16)
        w2h = np.ascontiguousarray(
            W2[el].reshape(EL, NF, 128, ND, 512).transpose(0, 3, 1, 2, 4)
        ).astype(ml_dtypes.bfloat16)
        b1h = np.ascontiguousarray(b1[el].reshape(EL, NF, 128).transpose(0, 2, 1))
        b2d = np.ascontiguousarray(b2[el]).astype(ml_dtypes.bfloat16)
        basev = np.full((128, 1), 1280.0 * c, np.float32)
        xtloc = np.ascontiguousarray(xtf[:, 512 * c:512 * (c + 1)])
        in_maps.append(dict(xtloc=xtloc, xb=xb, wg=Wg, w1h=w1h, w2h=w2h, b1h=b1h,
                            b2d=b2d, tokids=tokids, iota16=iota16,
                            onescol=onescol, tri128=tri128, identb=identb,
                            basev=basev))
    return in_maps


def _run(inputs, trace=False, trace_cores=None):
    if "nc" not in _CACHE:
        _CACHE["nc"] = _build_nc()
    nc = _CACHE["nc"]
    in_maps = _prep_inputs(inputs["x"], inputs["Wg"], inputs["W1"],
                           inputs["b1"], inputs["W2"], inputs["b2"])
    res = run_bass_kernel_spmd(nc, in_maps, list(range(8)), trace=trace,
                               trace_cores=trace_cores)
    y = np.zeros((T, D), np.float64)
    for r in res.results:
        y += r["y"].astype(np.float64)
    y = y.astype(np.float32).reshape(B, S, D)
    return y, res


def kernel(x, Wg, W1, b1, W2, b2):
    y, _ = _run(dict(x=x, Wg=Wg, W1=W1, b1=b1, W2=W2, b2=b2))
    return y
